# revision 8
# baseline (speedup 1.0000x reference)
"""Multi-head attention block (B=8, N=1024, H=8, d=128, D_in=256) on 8 trn2 cores.

Sharding: data-parallel over batch — core b computes batch element b entirely
(8 heads), no collectives. Host pre-transposes x and pre-scales wq by
1/sqrt(d); the additive [N,N] bias is shipped as exp(B)^T so the device does
exp(S+B) = exp(S) * expB with element-wise engines instead of an
identity-matmul PSUM preload (saves 65k PE rows).

Per-core dataflow (all matmuls float32r, moving free dim 512):
  QT[c,n], KT[c,n] = w.T @ x.T    (c-major so head slices are partition chunks)
  V[n,c]          = x @ wv        (n-major so PV stationary is a natural slice)
  head loop over 16 blocks t=(h,half), software-pipelined at m-granularity:
    S_T[m,n] = KT_h[d,m].T @ QT_h[d,n]     (single matmul per tile)
    at       = exp(S_T) * expB_T[m,n]      (ACT exp; mul split GPSIMD/DVE)
    rs[1,n]  = ones.T @ at                 (softmax denominator, PSUM-accum)
    pv[d,n]  = V_h[m,d].T @ at             (unnormalized, PSUM-accum)
    drain: recip -> DRAM-roundtrip partition-broadcast -> oh = pv * bc
    pj[j,n]  = pw_h[c,j].T @ oh ; yacc += pj
  yT = yacc + proj_b -> DRAM [128, 1024]; host transposes back.

Blocks are half-heads so each drain (DMA-latency-bound) hides behind the next
block's PE work. Emission order per block t: oh-mul(t-2) first (DVE), then the
m-loop [S(t,m) + ones/pv(t-1,m)], then recip/bcast(t-1), then pj/yacc(t-2) —
so the PE never queues behind a DMA-latency-bound op. PSUM: S/pj pool 3,
PV 3, RS 2 banks. QKV-projection setup is woven into blocks 0-7.
"""

import math
import sys

import numpy as np

if "/opt/trn_rl_repo" not in sys.path:
    sys.path.insert(0, "/opt/trn_rl_repo")

import ml_dtypes
import concourse.bass as bass
import concourse.tile as tile
from concourse import bacc
from concourse import mybir

F32 = mybir.dt.float32
F32R = mybir.dt.float32r
BF16 = mybir.dt.bfloat16
EXP = mybir.ActivationFunctionType.Exp
IDENT = mybir.ActivationFunctionType.Identity

N = 1024          # sequence length
D_IN = 256        # input dim
H = 8             # heads
DH = 128          # head dim
C = H * DH        # 1024
NCORES = 8
HALF = 512        # matmul moving free dim
NBLK = 16         # (head, half) blocks
POOL_MULS = 3     # expB muls per block routed to gpsimd (rest on DVE)


def build_nc():
    nc = bacc.Bacc("TRN2", target_bir_lowering=False, debug=False,
                   num_devices=NCORES)

    xT = nc.dram_tensor("xT", [D_IN, N], F32R, kind="ExternalInput").ap()
    eb = nc.dram_tensor("eb", [N, N], BF16, kind="ExternalInput").ap()
    wq = nc.dram_tensor("wq", [D_IN, C], F32R, kind="ExternalInput").ap()
    wk = nc.dram_tensor("wk", [D_IN, C], F32R, kind="ExternalInput").ap()
    wv = nc.dram_tensor("wv", [D_IN, C], F32R, kind="ExternalInput").ap()
    wqb = nc.dram_tensor("wqb", [128, 8], F32, kind="ExternalInput").ap()
    wkb = nc.dram_tensor("wkb", [128, 8], F32, kind="ExternalInput").ap()
    wvbb = nc.dram_tensor("wvbb", [128, C], F32, kind="ExternalInput").ap()
    pw = nc.dram_tensor("pw", [C, DH], F32R, kind="ExternalInput").ap()
    pb = nc.dram_tensor("pb", [128, 1], F32, kind="ExternalInput").ap()
    yT = nc.dram_tensor("yT", [DH, N], F32, kind="ExternalOutput").ap()

    with tile.TileContext(nc) as tc:
        build_body(nc, tc, xT, eb, wq, wk, wv, wqb, wkb, wvbb, pw, pb, yT)
    nc.compile()
    return nc


def build_body(nc, tc, xT, eb, wq, wk, wv, wqb, wkb, wvbb, pw, pb, yT):
    with (
        tc.tile_pool(name="persist", bufs=1) as P,
        tc.tile_pool(name="at", bufs=11) as AT,
        tc.tile_pool(name="oh", bufs=3) as OH,
        tc.tile_pool(name="bc", bufs=3) as BC,
        tc.tile_pool(name="rc", bufs=3) as RC,
        tc.tile_pool(name="dram", bufs=3, space="DRAM") as DR,
        tc.tile_pool(name="ps_s", bufs=3, space="PSUM") as PS_S,
        tc.tile_pool(name="ps_pj", bufs=1, space="PSUM") as PS_PJ,
        tc.tile_pool(name="ps_pv", bufs=3, space="PSUM") as PS_PV,
        tc.tile_pool(name="ps_rs", bufs=1, space="PSUM") as PS_RS,
    ):
        # ---- input DMAs, bandwidth-priority order ----
        xt2 = P.tile([128, 2, N], F32R, tag="xt2")
        nc.sync.dma_start(out=xt2, in_=xT.rearrange("(a p) n -> p a n", p=128))
        w2 = {}
        for wname, wdram in (("wq", wq), ("wk", wk), ("wv", wv)):
            t = P.tile([128, 2, C], F32R, tag=wname, name=wname)
            nc.sync.dma_start(out=t,
                              in_=wdram.rearrange("(a p) c -> p a c", p=128))
            w2[wname] = t
        eb_sb = []
        for m in range(8):
            t = P.tile([128, N], BF16, tag=f"eb{m}", name=f"ebl{m}")
            nc.sync.dma_start(out=t, in_=eb[m * 128:(m + 1) * 128, :])
            eb_sb.append(t)
        wqb_sb = P.tile([128, 8], F32, tag="wqb")
        nc.sync.dma_start(out=wqb_sb, in_=wqb)
        wkb_sb = P.tile([128, 8], F32, tag="wkb")
        nc.sync.dma_start(out=wkb_sb, in_=wkb)
        wvbb_sb = P.tile([128, C], F32, tag="wvbb")
        nc.sync.dma_start(out=wvbb_sb, in_=wvbb)
        pw_sb = P.tile([128, 8, 128], F32R, tag="pw")
        nc.sync.dma_start(out=pw_sb, in_=pw.rearrange("(a p) j -> p a j", p=128))
        pb_sb = P.tile([128, 1], F32, tag="pb")
        nc.sync.dma_start(out=pb_sb, in_=pb)

        # ---- persistent tiles ----
        ones = P.tile([128, 1], BF16, tag="ones")
        with tc.tile_pool(name="mkconst", bufs=1) as MK:
            ones_f = MK.tile([128, 1], F32, tag="ones_f")
            nc.vector.memset(ones_f, 1.0)
            nc.vector.tensor_copy(ones, ones_f)
        qt_sb = [P.tile([128, N], F32R, tag=f"qt{c}", name=f"qt{c}") for c in range(8)]
        kt_sb = [P.tile([128, N], F32R, tag=f"kt{c}", name=f"kt{c}") for c in range(8)]
        v_sb = [P.tile([128, C], BF16, tag=f"v{n}", name=f"v{n}") for n in range(8)]
        yacc = P.tile([128, N], F32, tag="yacc")
        yt_sb = P.tile([128, N], F32, tag="yt")

        # ---- setup pieces (emitted interleaved into early blocks) ----
        def qkt_piece(wname, b_sb, dst, c, on_act):
            cs = slice(c * 128, (c + 1) * 128)
            for i in range(2):
                ns = slice(i * HALF, (i + 1) * HALF)
                ps = PS_S.tile([128, HALF], F32)
                nc.tensor.matmul(ps, w2[wname][:, 0, cs], xt2[:, 0, ns],
                                 start=True, stop=False)
                nc.tensor.matmul(ps, w2[wname][:, 1, cs], xt2[:, 1, ns],
                                 start=False, stop=True)
                if on_act:
                    nc.scalar.activation(dst[c][:, ns], ps, func=IDENT,
                                         bias=b_sb[:, c:c + 1])
                else:
                    nc.vector.tensor_scalar_add(dst[c][:, ns], ps,
                                                b_sb[:, c:c + 1])

        def v_piece(n):
            nsl = slice(n * 128, (n + 1) * 128)
            for i in range(2):
                cs = slice(i * HALF, (i + 1) * HALF)
                ps = PS_S.tile([128, HALF], F32)
                nc.tensor.matmul(ps, xt2[:, 0, nsl], w2["wv"][:, 0, cs],
                                 start=True, stop=False)
                nc.tensor.matmul(ps, xt2[:, 1, nsl], w2["wv"][:, 1, cs],
                                 start=False, stop=True)
                nc.vector.tensor_add(v_sb[n][:, cs], ps, wvbb_sb[:, cs])

        # qt/kt c0 first so block 0's S matmuls can start immediately
        qkt_piece("wq", wqb_sb, qt_sb, 0, True)
        qkt_piece("wk", wkb_sb, kt_sb, 0, False)
        # remaining pieces woven into blocks: V into block 0 (needed by the
        # first ones/pv in block 1), qt/kt chunk c before block 2c
        pieces = [lambda n=n: v_piece(n) for n in range(8)]
        for c in range(1, 8):
            pieces.append(lambda c=c: qkt_piece("wq", wqb_sb, qt_sb, c, True))
            pieces.append(lambda c=c: qkt_piece("wk", wkb_sb, kt_sb, c, False))
        piece_quota = {0: 8, 1: 2, 2: 2, 3: 2, 4: 2, 5: 2, 6: 2, 7: 2}

        # ---- pipelined block loop: block t = (head h, n-half i) ----
        at_t = {}     # (t, m) -> at tile
        pv_t = {}     # t -> pv psum tile
        rs_t = {}     # t -> rowsum psum tile
        bc_t = {}     # t -> broadcast recip tile
        oh_t = {}     # t -> normalized head-output tile

        def s_exp_mul(t, m):
            h, i = divmod(t, 2)
            ns = slice(i * HALF, (i + 1) * HALF)
            ms = slice(m * 128, (m + 1) * 128)
            ps = PS_S.tile([128, HALF], F32)
            nc.tensor.matmul(ps, kt_sb[h][:, ms], qt_sb[h][:, ns],
                             start=True, stop=True)
            at = AT.tile([128, HALF], BF16)
            nc.scalar.activation(at, ps, func=EXP)
            eng = nc.gpsimd if m < POOL_MULS else nc.vector
            eng.tensor_mul(at, at, eb_sb[m][:, ns])
            at_t[(t, m)] = at

        def ones_pv(t, m):
            h, _ = divmod(t, 2)
            hs = slice(h * 128, (h + 1) * 128)
            if m == 0:
                rs_t[t] = PS_RS.tile([1, HALF], F32, tag="rs", name=f"rs{t}")
                pv_t[t] = PS_PV.tile([128, HALF], F32, tag="pv", name=f"pv{t}")
            at = at_t.pop((t, m))
            nc.tensor.matmul(rs_t[t], ones, at, start=(m == 0), stop=(m == 7))
            nc.tensor.matmul(pv_t[t], v_sb[m][:, hs], at,
                             start=(m == 0), stop=(m == 7))

        def drain_start(t):
            # softmax denominators: reciprocal + partition-broadcast roundtrip
            rc = RC.tile([1, HALF], F32, tag="rc", name=f"rc{t}")
            nc.vector.reciprocal(rc, rs_t.pop(t))
            scratch = DR.tile([HALF], F32, name=f"scr{t}")
            nc.sync.dma_start(out=scratch, in_=rc)
            bc = BC.tile([128, HALF], F32, tag="bc", name=f"bc{t}")
            nc.sync.dma_start(out=bc, in_=scratch.partition_broadcast(128))
            bc_t[t] = bc

        def oh_mul(t):
            oh = OH.tile([128, HALF], F32R, tag="oh", name=f"oh{t}")
            nc.vector.tensor_mul(oh, pv_t.pop(t), bc_t.pop(t))
            oh_t[t] = oh

        def proj_acc(t):
            h, i = divmod(t, 2)
            ns = slice(i * HALF, (i + 1) * HALF)
            pj = PS_PJ.tile([128, HALF], F32, tag="pj", name=f"pj{t}")
            nc.tensor.matmul(pj, pw_sb[:, h, :], oh_t.pop(t),
                             start=True, stop=True)
            if h == 0:
                nc.vector.tensor_copy(yacc[:, ns], pj)
            else:
                nc.vector.tensor_add(yacc[:, ns], yacc[:, ns], pj)

        def finalize(i):
            ns = slice(i * HALF, (i + 1) * HALF)
            nc.scalar.activation(yt_sb[:, ns], yacc[:, ns], func=IDENT,
                                 bias=pb_sb)
            nc.sync.dma_start(out=yT[:, ns], in_=yt_sb[:, ns])

        pi = 0
        for t in range(NBLK + 2):
            if 2 <= t:
                oh_mul(t - 2)     # first in this block's DVE queue: frees pv
            quota = piece_quota.get(t, 0)
            for m in range(8):
                if t < NBLK:
                    s_exp_mul(t, m)
                if 1 <= t <= NBLK:
                    ones_pv(t - 1, m)
                if quota and m % (8 // quota) == (8 // quota) - 1:
                    pieces[pi](); pi += 1
            if 1 <= t <= NBLK:
                drain_start(t - 1)   # rs(t-1) just stopped; launch DMA chain
            if 2 <= t:
                proj_acc(t - 2)      # PE reaches this after the block's work
                if t - 2 >= NBLK - 2:
                    finalize((t - 2) % 2)
        assert pi == len(pieces)


_CACHE = {}


def _prep_inputs(x, B_bias, wq_w, wq_b, wk_w, wk_b, wv_w, wv_b, proj_w, proj_b):
    s = 1.0 / math.sqrt(DH)
    f = np.float32
    xTh = np.ascontiguousarray(x.transpose(0, 2, 1)).astype(f)      # [8,256,1024]
    ebh = np.exp(np.asarray(B_bias, np.float32).T).astype(ml_dtypes.bfloat16)
    wq_s = (np.asarray(wq_w) * s).astype(f)
    wqb_t = np.ascontiguousarray((np.asarray(wq_b) * s).reshape(8, 128).T)
    wkb_t = np.ascontiguousarray(np.asarray(wk_b, f).reshape(8, 128).T)
    wvbb = np.ascontiguousarray(np.broadcast_to(np.asarray(wv_b, f), (128, C)))
    pb_t = np.ascontiguousarray(np.asarray(proj_b, f).reshape(128, 1))
    shared = dict(eb=ebh, wq=wq_s, wk=np.asarray(wk_w, f),
                  wv=np.asarray(wv_w, f), wqb=wqb_t, wkb=wkb_t, wvbb=wvbb,
                  pw=np.asarray(proj_w, f), pb=pb_t)
    return [dict(shared, xT=xTh[b]) for b in range(NCORES)]


def kernel(**inputs):
    from concourse.bass_utils import run_bass_kernel_spmd

    if "nc" not in _CACHE:
        _CACHE["nc"] = build_nc()
    nc = _CACHE["nc"]
    in_maps = _prep_inputs(**inputs)
    res = run_bass_kernel_spmd(nc, in_maps, core_ids=list(range(NCORES)))
    out = np.stack([np.asarray(res.results[b]["yT"]).T for b in range(NCORES)])
    return np.ascontiguousarray(out.astype(np.float32))


# revision 9
# speedup vs baseline: 1.3233x; 1.3233x over previous
"""Multi-head attention block (B=8, N=1024, H=8, d=128, D_in=256) on 8 trn2 cores.

Sharding: data-parallel over batch — core b computes batch element b entirely
(8 heads), no collectives. Host pre-transposes x and pre-scales wq by
1/sqrt(d); the additive [N,N] bias is shipped as exp(B)^T so the device does
exp(S+B) = exp(S) * expB with element-wise engines instead of an
identity-matmul PSUM preload (saves 65k PE rows).

Per-core dataflow (all matmuls float32r, moving free dim 512):
  QT[c,n], KT[c,n] = w.T @ x.T    (c-major so head slices are partition chunks)
  V[n,c]          = x @ wv        (n-major so PV stationary is a natural slice)
  head loop over 16 blocks t=(h,half), software-pipelined at m-granularity:
    S_T[m,n] = KT_h[d,m].T @ QT_h[d,n]     (single matmul per tile)
    at       = exp(S_T) * expB_T[m,n]      (ACT exp; mul split GPSIMD/DVE)
    rs[1,n]  = ones.T @ at                 (softmax denominator, PSUM-accum)
    pv[d,n]  = V_h[m,d].T @ at             (unnormalized, PSUM-accum)
    drain: recip -> DRAM-roundtrip partition-broadcast -> oh = pv * bc
    pj[j,n]  = pw_h[c,j].T @ oh ; yacc += pj
  yT = yacc + proj_b -> DRAM [128, 1024]; host transposes back.

Blocks are half-heads so each drain (DMA-latency-bound) hides behind the next
block's PE work. Emission order per block t: oh-mul(t-2) first (DVE), then the
m-loop [S(t,m) + ones/pv(t-1,m)], then recip/bcast(t-1), then pj/yacc(t-2) —
so the PE never queues behind a DMA-latency-bound op. PSUM: S/pj pool 3,
PV 3, RS 2 banks. QKV-projection setup is woven into blocks 0-7.
"""

import math
import sys

import numpy as np

if "/opt/trn_rl_repo" not in sys.path:
    sys.path.insert(0, "/opt/trn_rl_repo")

import ml_dtypes
import concourse.bass as bass
import concourse.tile as tile
from concourse import bacc
from concourse import mybir

F32 = mybir.dt.float32
F32R = mybir.dt.float32r
BF16 = mybir.dt.bfloat16
EXP = mybir.ActivationFunctionType.Exp
IDENT = mybir.ActivationFunctionType.Identity

N = 1024          # sequence length
D_IN = 256        # input dim
H = 8             # heads
DH = 128          # head dim
C = H * DH        # 1024
NCORES = 8
HALF = 512        # matmul moving free dim
NBLK = 16         # (head, half) blocks
POOL_MULS = 3     # expB muls per block routed to gpsimd (rest on DVE)


def build_nc():
    nc = bacc.Bacc("TRN2", target_bir_lowering=False, debug=False,
                   num_devices=NCORES)

    xT = nc.dram_tensor("xT", [D_IN, N], F32R, kind="ExternalInput").ap()
    eb = nc.dram_tensor("eb", [N, N], F32, kind="ExternalInput").ap()
    wq = nc.dram_tensor("wq", [D_IN, C], F32R, kind="ExternalInput").ap()
    wk = nc.dram_tensor("wk", [D_IN, C], F32R, kind="ExternalInput").ap()
    wv = nc.dram_tensor("wv", [D_IN, C], F32R, kind="ExternalInput").ap()
    wqb = nc.dram_tensor("wqb", [128, 8], F32, kind="ExternalInput").ap()
    wkb = nc.dram_tensor("wkb", [128, 8], F32, kind="ExternalInput").ap()
    wvbb = nc.dram_tensor("wvbb", [128, C], F32, kind="ExternalInput").ap()
    pw = nc.dram_tensor("pw", [C, DH], F32R, kind="ExternalInput").ap()
    pb = nc.dram_tensor("pb", [128, 1], F32, kind="ExternalInput").ap()
    yT = nc.dram_tensor("yT", [DH, N], F32, kind="ExternalOutput").ap()

    with tile.TileContext(nc) as tc:
        build_body(nc, tc, xT, eb, wq, wk, wv, wqb, wkb, wvbb, pw, pb, yT)
    nc.compile()
    return nc


def build_body(nc, tc, xT, eb, wq, wk, wv, wqb, wkb, wvbb, pw, pb, yT):
    with (
        tc.tile_pool(name="persist", bufs=1) as P,
        tc.tile_pool(name="at", bufs=9) as AT,
        tc.tile_pool(name="oh", bufs=2) as OH,
        tc.tile_pool(name="bc", bufs=2) as BC,
        tc.tile_pool(name="rc", bufs=2) as RC,
        tc.tile_pool(name="dram", bufs=3, space="DRAM") as DR,
        tc.tile_pool(name="ps_s", bufs=3, space="PSUM") as PS_S,
        tc.tile_pool(name="ps_pj", bufs=1, space="PSUM") as PS_PJ,
        tc.tile_pool(name="ps_pv", bufs=3, space="PSUM") as PS_PV,
        tc.tile_pool(name="ps_rs", bufs=1, space="PSUM") as PS_RS,
    ):
        # ---- input DMAs, bandwidth-priority order ----
        def load2(wname, wdram):
            t = P.tile([128, 2, C], F32R, tag=wname, name=wname)
            nc.sync.dma_start(out=t,
                              in_=wdram.rearrange("(a p) c -> p a c", p=128))
            return t
        xt2 = P.tile([128, 2, N], F32R, tag="xt2")
        nc.sync.dma_start(out=xt2, in_=xT.rearrange("(a p) n -> p a n", p=128))
        w2 = {"wq": load2("wq", wq)}
        wqb_sb = P.tile([128, 8], F32, tag="wqb")
        nc.sync.dma_start(out=wqb_sb, in_=wqb)
        wkb_sb = P.tile([128, 8], F32, tag="wkb")
        nc.sync.dma_start(out=wkb_sb, in_=wkb)
        wvbb_sb = P.tile([128, C], F32, tag="wvbb")
        nc.sync.dma_start(out=wvbb_sb, in_=wvbb)
        pb_sb = P.tile([128, 1], F32, tag="pb")
        nc.sync.dma_start(out=pb_sb, in_=pb)
        w2["wk"] = load2("wk", wk)
        w2["wv"] = load2("wv", wv)
        pw_sb = P.tile([128, 8, 128], F32R, tag="pw")
        nc.sync.dma_start(out=pw_sb, in_=pw.rearrange("(a p) j -> p a j", p=128))
        eb_sb = []
        for m in range(8):
            t = P.tile([128, N], F32, tag=f"eb{m}", name=f"ebl{m}")
            nc.sync.dma_start(out=t, in_=eb[m * 128:(m + 1) * 128, :])
            eb_sb.append(t)

        # ---- persistent tiles ----
        ones = P.tile([128, 1], F32R, tag="ones")
        with tc.tile_pool(name="mkconst", bufs=1) as MK:
            ones_f = MK.tile([128, 1], F32, tag="ones_f")
            nc.vector.memset(ones_f, 1.0)
            nc.vector.tensor_copy(ones, ones_f)
            warm = MK.tile([128, 1], F32, tag="warm")
            nc.scalar.activation(warm, ones_f, func=EXP)
        qt_sb = [P.tile([128, N], F32R, tag=f"qt{c}", name=f"qt{c}") for c in range(8)]
        kt_sb = [P.tile([128, N], F32R, tag=f"kt{c}", name=f"kt{c}") for c in range(8)]
        v_sb = [P.tile([128, C], F32R, tag=f"v{n}", name=f"v{n}") for n in range(8)]
        yacc = P.tile([128, N], F32, tag="yacc")
        yt_sb = P.tile([128, N], F32, tag="yt")

        # ---- setup pieces (emitted interleaved into early blocks) ----
        def qkt_piece(wname, b_sb, dst, c, on_act):
            cs = slice(c * 128, (c + 1) * 128)
            for i in range(2):
                ns = slice(i * HALF, (i + 1) * HALF)
                ps = PS_S.tile([128, HALF], F32)
                nc.tensor.matmul(ps, w2[wname][:, 0, cs], xt2[:, 0, ns],
                                 start=True, stop=False)
                nc.tensor.matmul(ps, w2[wname][:, 1, cs], xt2[:, 1, ns],
                                 start=False, stop=True)
                if on_act:
                    nc.scalar.activation(dst[c][:, ns], ps, func=IDENT,
                                         bias=b_sb[:, c:c + 1])
                else:
                    nc.vector.tensor_scalar_add(dst[c][:, ns], ps,
                                                b_sb[:, c:c + 1])

        def v_piece(n):
            nsl = slice(n * 128, (n + 1) * 128)
            for i in range(2):
                cs = slice(i * HALF, (i + 1) * HALF)
                ps = PS_S.tile([128, HALF], F32)
                nc.tensor.matmul(ps, xt2[:, 0, nsl], w2["wv"][:, 0, cs],
                                 start=True, stop=False)
                nc.tensor.matmul(ps, xt2[:, 1, nsl], w2["wv"][:, 1, cs],
                                 start=False, stop=True)
                nc.vector.tensor_add(v_sb[n][:, cs], ps, wvbb_sb[:, cs])

        # qt/kt c0 first so block 0's S matmuls can start immediately
        qkt_piece("wq", wqb_sb, qt_sb, 0, True)
        qkt_piece("wk", wkb_sb, kt_sb, 0, False)
        # remaining pieces woven into blocks: V into block 0 (needed by the
        # first ones/pv in block 1), qt/kt chunk c before block 2c
        pieces = [lambda n=n: v_piece(n) for n in range(8)]
        for c in range(1, 8):
            pieces.append(lambda c=c: qkt_piece("wq", wqb_sb, qt_sb, c, True))
            pieces.append(lambda c=c: qkt_piece("wk", wkb_sb, kt_sb, c, False))
        piece_quota = {0: 8, 1: 2, 2: 2, 3: 2, 4: 2, 5: 2, 6: 2, 7: 2}

        # ---- pipelined block loop: block t = (head h, n-half i) ----
        at_t = {}     # (t, m) -> at tile
        pv_t = {}     # t -> pv psum tile
        rs_t = {}     # t -> rowsum psum tile
        bc_t = {}     # t -> broadcast recip tile
        oh_t = {}     # t -> normalized head-output tile

        def s_exp_mul(t, m):
            h, i = divmod(t, 2)
            ns = slice(i * HALF, (i + 1) * HALF)
            ms = slice(m * 128, (m + 1) * 128)
            ps = PS_S.tile([128, HALF], F32)
            nc.tensor.matmul(ps, kt_sb[h][:, ms], qt_sb[h][:, ns],
                             start=True, stop=True)
            at = AT.tile([128, HALF], F32R)
            nc.scalar.activation(at, ps, func=EXP)
            eng = nc.gpsimd if m < POOL_MULS else nc.vector
            eng.tensor_mul(at, at, eb_sb[m][:, ns])
            at_t[(t, m)] = at

        def ones_pv(t, m):
            h, _ = divmod(t, 2)
            hs = slice(h * 128, (h + 1) * 128)
            if m == 0:
                rs_t[t] = PS_RS.tile([1, HALF], F32, tag="rs", name=f"rs{t}")
                pv_t[t] = PS_PV.tile([128, HALF], F32, tag="pv", name=f"pv{t}")
            at = at_t.pop((t, m))
            nc.tensor.matmul(rs_t[t], ones, at, start=(m == 0), stop=(m == 7))
            nc.tensor.matmul(pv_t[t], v_sb[m][:, hs], at,
                             start=(m == 0), stop=(m == 7))

        def drain_start(t):
            # softmax denominators: reciprocal + partition-broadcast roundtrip
            rc = RC.tile([1, HALF], F32, tag="rc", name=f"rc{t}")
            nc.vector.reciprocal_approx_fast(out=rc, in_=rs_t.pop(t))
            scratch = DR.tile([HALF], F32, name=f"scr{t}")
            nc.sync.dma_start(out=scratch, in_=rc)
            bc = BC.tile([128, HALF], F32, tag="bc", name=f"bc{t}")
            nc.sync.dma_start(out=bc, in_=scratch.partition_broadcast(128))
            bc_t[t] = bc

        def oh_mul(t):
            oh = OH.tile([128, HALF], F32R, tag="oh", name=f"oh{t}")
            nc.vector.tensor_mul(oh, pv_t.pop(t), bc_t.pop(t))
            oh_t[t] = oh

        def proj_acc(t):
            h, i = divmod(t, 2)
            ns = slice(i * HALF, (i + 1) * HALF)
            pj = PS_PJ.tile([128, HALF], F32, tag="pj", name=f"pj{t}")
            nc.tensor.matmul(pj, pw_sb[:, h, :], oh_t.pop(t),
                             start=True, stop=True)
            if h == 0:
                nc.vector.tensor_copy(yacc[:, ns], pj)
            else:
                nc.vector.tensor_add(yacc[:, ns], yacc[:, ns], pj)

        def finalize(i):
            ns = slice(i * HALF, (i + 1) * HALF)
            nc.scalar.activation(yt_sb[:, ns], yacc[:, ns], func=IDENT,
                                 bias=pb_sb)
            nc.sync.dma_start(out=yT[:, ns], in_=yt_sb[:, ns])

        pi = 0
        for t in range(NBLK + 2):
            quota = piece_quota.get(t, 0)
            for m in range(8):
                if t < NBLK:
                    s_exp_mul(t, m)
                if 1 <= t <= NBLK:
                    ones_pv(t - 1, m)
                if quota and m % (8 // quota) == (8 // quota) - 1:
                    pieces[pi](); pi += 1
            if 1 <= t <= NBLK:
                drain_start(t - 1)   # rs(t-1) just stopped; launch DMA chain
            if 2 <= t:
                oh_mul(t - 2)        # bc(t-2) arrived during this block
                proj_acc(t - 2)      # PE reaches this after the block's work
                if t - 2 >= NBLK - 2:
                    finalize((t - 2) % 2)
        assert pi == len(pieces)


_CACHE = {}


def _prep_inputs(x, B_bias, wq_w, wq_b, wk_w, wk_b, wv_w, wv_b, proj_w, proj_b):
    s = 1.0 / math.sqrt(DH)
    f = np.float32
    xTh = np.ascontiguousarray(x.transpose(0, 2, 1)).astype(f)      # [8,256,1024]
    ebh = np.ascontiguousarray(np.exp(np.asarray(B_bias, np.float32).T))
    wq_s = (np.asarray(wq_w) * s).astype(f)
    wqb_t = np.ascontiguousarray((np.asarray(wq_b) * s).reshape(8, 128).T)
    wkb_t = np.ascontiguousarray(np.asarray(wk_b, f).reshape(8, 128).T)
    wvbb = np.ascontiguousarray(np.broadcast_to(np.asarray(wv_b, f), (128, C)))
    pb_t = np.ascontiguousarray(np.asarray(proj_b, f).reshape(128, 1))
    shared = dict(eb=ebh, wq=wq_s, wk=np.asarray(wk_w, f),
                  wv=np.asarray(wv_w, f), wqb=wqb_t, wkb=wkb_t, wvbb=wvbb,
                  pw=np.asarray(proj_w, f), pb=pb_t)
    return [dict(shared, xT=xTh[b]) for b in range(NCORES)]


def kernel(**inputs):
    from concourse.bass_utils import run_bass_kernel_spmd

    if "nc" not in _CACHE:
        _CACHE["nc"] = build_nc()
    nc = _CACHE["nc"]
    in_maps = _prep_inputs(**inputs)
    res = run_bass_kernel_spmd(nc, in_maps, core_ids=list(range(NCORES)))
    out = np.stack([np.asarray(res.results[b]["yT"]).T for b in range(NCORES)])
    return np.ascontiguousarray(out.astype(np.float32))


# revision 12
# speedup vs baseline: 1.3422x; 1.0143x over previous
"""Multi-head attention block (B=8, N=1024, H=8, d=128, D_in=256) on 8 trn2 cores.

Sharding: data-parallel over batch — core b computes batch element b entirely
(8 heads), no collectives. Host pre-transposes x and pre-scales wq by
1/sqrt(d); the additive [N,N] bias is shipped as exp(B)^T so the device does
exp(S+B) = exp(S) * expB with element-wise engines instead of an
identity-matmul PSUM preload (saves 65k PE rows).

Per-core dataflow (all matmuls float32r, moving free dim 512):
  QT[c,n], KT[c,n] = w.T @ x.T    (c-major so head slices are partition chunks)
  V[n,c]          = x @ wv        (n-major so PV stationary is a natural slice)
  head loop over 16 blocks t=(h,half), software-pipelined at m-granularity:
    S_T[m,n] = KT_h[d,m].T @ QT_h[d,n]     (single matmul per tile)
    at       = exp(S_T) * expB_T[m,n]      (ACT exp; mul split GPSIMD/DVE)
    rs[1,n]  = ones.T @ at                 (softmax denominator, PSUM-accum)
    pv[d,n]  = V_h[m,d].T @ at             (unnormalized, PSUM-accum)
    drain: recip -> DRAM-roundtrip partition-broadcast -> oh = pv * bc
    pj[j,n]  = pw_h[c,j].T @ oh ; yacc += pj
  yT = yacc + proj_b -> DRAM [128, 1024]; host transposes back.

Blocks are half-heads so each drain (DMA-latency-bound) hides behind the next
block's PE work. Emission order per block t: oh-mul(t-2) first (DVE), then the
m-loop [S(t,m) + ones/pv(t-1,m)], then recip/bcast(t-1), then pj/yacc(t-2) —
so the PE never queues behind a DMA-latency-bound op. PSUM: S/pj pool 3,
PV 3, RS 2 banks. QKV-projection setup is woven into blocks 0-7.
"""

import math
import sys

import numpy as np

if "/opt/trn_rl_repo" not in sys.path:
    sys.path.insert(0, "/opt/trn_rl_repo")

import ml_dtypes
import concourse.bass as bass
import concourse.tile as tile
from concourse import bacc
from concourse import mybir

F32 = mybir.dt.float32
F32R = mybir.dt.float32r
BF16 = mybir.dt.bfloat16
EXP = mybir.ActivationFunctionType.Exp
IDENT = mybir.ActivationFunctionType.Identity

N = 1024          # sequence length
D_IN = 256        # input dim
H = 8             # heads
DH = 128          # head dim
C = H * DH        # 1024
NCORES = 8
HALF = 512        # matmul moving free dim
NBLK = 16         # (head, half) blocks
POOL_MULS = 3     # expB muls per block routed to gpsimd (rest on DVE)


def build_nc():
    nc = bacc.Bacc("TRN2", target_bir_lowering=False, debug=False,
                   num_devices=NCORES)

    xT = nc.dram_tensor("xT", [D_IN, N], F32R, kind="ExternalInput").ap()
    eb = nc.dram_tensor("eb", [N, N], F32, kind="ExternalInput").ap()
    wq = nc.dram_tensor("wq", [D_IN, C], F32R, kind="ExternalInput").ap()
    wk = nc.dram_tensor("wk", [D_IN, C], F32R, kind="ExternalInput").ap()
    wv = nc.dram_tensor("wv", [D_IN, C], F32R, kind="ExternalInput").ap()
    wqb = nc.dram_tensor("wqb", [128, 8], F32, kind="ExternalInput").ap()
    wkb = nc.dram_tensor("wkb", [128, 8], F32, kind="ExternalInput").ap()
    wvbb = nc.dram_tensor("wvbb", [128, C], F32, kind="ExternalInput").ap()
    pw = nc.dram_tensor("pw", [C, DH], F32R, kind="ExternalInput").ap()
    pb = nc.dram_tensor("pb", [128, 1], F32, kind="ExternalInput").ap()
    yT = nc.dram_tensor("yT", [DH, N], F32, kind="ExternalOutput").ap()

    with tile.TileContext(nc) as tc:
        build_body(nc, tc, xT, eb, wq, wk, wv, wqb, wkb, wvbb, pw, pb, yT)
    nc.compile()
    return nc


def build_body(nc, tc, xT, eb, wq, wk, wv, wqb, wkb, wvbb, pw, pb, yT):
    with (
        tc.tile_pool(name="persist", bufs=1) as P,
        tc.tile_pool(name="at", bufs=9) as AT,
        tc.tile_pool(name="oh", bufs=2) as OH,
        tc.tile_pool(name="rc", bufs=2) as RC,
        tc.tile_pool(name="bcs", bufs=2) as BCS,
        tc.tile_pool(name="ps_s", bufs=3, space="PSUM") as PS_S,
        tc.tile_pool(name="ps_pj", bufs=1, space="PSUM") as PS_PJ,
        tc.tile_pool(name="ps_bc", bufs=1, space="PSUM") as PS_BC,
        tc.tile_pool(name="ps_pv", bufs=2, space="PSUM") as PS_PV,
        tc.tile_pool(name="ps_rs", bufs=1, space="PSUM") as PS_RS,
    ):
        # ---- input DMAs, bandwidth-priority order ----
        def load2(wname, wdram, c0, c1):
            t = P.tile([128, 2, c1 - c0], F32R, tag=wname, name=wname)
            nc.sync.dma_start(out=t, in_=wdram[:, c0:c1].rearrange(
                "(a p) c -> p a c", p=128))
            return t
        xt = [P.tile([128, N], F32R, tag=f"xt{d}", name=f"xt{d}")
              for d in range(2)]
        for d in range(2):
            nc.sync.dma_start(out=xt[d], in_=xT[d * 128:(d + 1) * 128, :])
        wq_c0 = load2("wqc0", wq, 0, 128)
        wk_c0 = load2("wkc0", wk, 0, 128)
        wqb_sb = P.tile([128, 8], F32, tag="wqb")
        nc.sync.dma_start(out=wqb_sb, in_=wqb)
        wkb_sb = P.tile([128, 8], F32, tag="wkb")
        nc.sync.dma_start(out=wkb_sb, in_=wkb)
        wvbb_sb = P.tile([128, C], F32, tag="wvbb")
        nc.sync.dma_start(out=wvbb_sb, in_=wvbb)
        pb_sb = P.tile([128, 1], F32, tag="pb")
        nc.sync.dma_start(out=pb_sb, in_=pb)
        w2 = {"wq": load2("wq", wq, 128, C), "wk": load2("wk", wk, 128, C),
              "wv": load2("wv", wv, 0, C)}
        pw_sb = P.tile([128, 8, 128], F32R, tag="pw")
        nc.sync.dma_start(out=pw_sb, in_=pw.rearrange("(a p) j -> p a j", p=128))
        eb_sb = []
        for m in range(8):
            t = P.tile([128, N], F32, tag=f"eb{m}", name=f"ebl{m}")
            nc.sync.dma_start(out=t, in_=eb[m * 128:(m + 1) * 128, :])
            eb_sb.append(t)

        # ---- persistent tiles ----
        ones = P.tile([128, 1], F32R, tag="ones")
        ones_row = P.tile([1, 128], F32R, tag="ones_row")
        with tc.tile_pool(name="mkconst", bufs=1) as MK:
            ones_f = MK.tile([128, 1], F32, tag="ones_f")
            nc.vector.memset(ones_f, 1.0)
            nc.vector.tensor_copy(ones, ones_f)
            warm = MK.tile([128, 1], F32, tag="warm")
            nc.scalar.activation(warm, ones_f, func=EXP)
            onesr_f = MK.tile([1, 128], F32, tag="onesr_f")
            nc.vector.memset(onesr_f, 1.0)
            nc.vector.tensor_copy(ones_row, onesr_f)
        qt_sb = [P.tile([128, N], F32R, tag=f"qt{c}", name=f"qt{c}") for c in range(8)]
        kt_sb = [P.tile([128, N], F32R, tag=f"kt{c}", name=f"kt{c}") for c in range(8)]
        v_sb = [P.tile([128, C], F32R, tag=f"v{n}", name=f"v{n}") for n in range(8)]
        yacc = P.tile([128, N], F32, tag="yacc")
        yt_sb = P.tile([128, N], F32, tag="yt")

        # ---- setup pieces (emitted interleaved into early blocks) ----
        def qkt_piece(wname, b_sb, dst, c, on_act):
            if c == 0:
                wt = wq_c0 if wname == "wq" else wk_c0
                cs = slice(0, 128)
            else:
                wt = w2[wname]
                cs = slice((c - 1) * 128, c * 128)
            for i in range(2):
                ns = slice(i * HALF, (i + 1) * HALF)
                ps = PS_S.tile([128, HALF], F32)
                nc.tensor.matmul(ps, wt[:, 0, cs], xt[0][:, ns],
                                 start=True, stop=False)
                nc.tensor.matmul(ps, wt[:, 1, cs], xt[1][:, ns],
                                 start=False, stop=True)
                if on_act:
                    nc.scalar.activation(dst[c][:, ns], ps, func=IDENT,
                                         bias=b_sb[:, c:c + 1])
                else:
                    nc.vector.tensor_scalar_add(dst[c][:, ns], ps,
                                                b_sb[:, c:c + 1])

        def v_piece(n):
            nsl = slice(n * 128, (n + 1) * 128)
            for i in range(2):
                cs = slice(i * HALF, (i + 1) * HALF)
                ps = PS_S.tile([128, HALF], F32)
                nc.tensor.matmul(ps, xt[0][:, nsl], w2["wv"][:, 0, cs],
                                 start=True, stop=False)
                nc.tensor.matmul(ps, xt[1][:, nsl], w2["wv"][:, 1, cs],
                                 start=False, stop=True)
                nc.vector.tensor_add(v_sb[n][:, cs], ps, wvbb_sb[:, cs])

        # qt/kt c0 first so block 0's S matmuls can start immediately
        qkt_piece("wq", wqb_sb, qt_sb, 0, True)
        qkt_piece("wk", wkb_sb, kt_sb, 0, False)
        # remaining pieces woven into blocks: V into block 0 (needed by the
        # first ones/pv in block 1), qt/kt chunk c before block 2c
        pieces = [lambda n=n: v_piece(n) for n in range(8)]
        for c in range(1, 8):
            pieces.append(lambda c=c: qkt_piece("wq", wqb_sb, qt_sb, c, True))
            pieces.append(lambda c=c: qkt_piece("wk", wkb_sb, kt_sb, c, False))
        piece_quota = {0: 8, 1: 2, 2: 2, 3: 2, 4: 2, 5: 2, 6: 2, 7: 2}

        # ---- pipelined block loop: block t = (head h, n-half i) ----
        at_t = {}     # (t, m) -> at tile
        pv_t = {}     # t -> pv psum tile
        rs_t = {}     # t -> rowsum psum tile
        rc_t = {}     # t -> reciprocal rowsum [1, HALF]
        bcp_t = {}    # t -> PE-broadcast recip psum tile
        oh_t = {}     # t -> normalized head-output tile

        def s_exp_mul(t, m):
            h, i = divmod(t, 2)
            ns = slice(i * HALF, (i + 1) * HALF)
            ms = slice(m * 128, (m + 1) * 128)
            ps = PS_S.tile([128, HALF], F32)
            nc.tensor.matmul(ps, kt_sb[h][:, ms], qt_sb[h][:, ns],
                             start=True, stop=True)
            at = AT.tile([128, HALF], F32R)
            nc.scalar.activation(at, ps, func=EXP)
            eng = nc.gpsimd if m < POOL_MULS else nc.vector
            eng.tensor_mul(at, at, eb_sb[m][:, ns])
            at_t[(t, m)] = at

        def ones_pv(t, m):
            h, _ = divmod(t, 2)
            hs = slice(h * 128, (h + 1) * 128)
            if m == 0:
                rs_t[t] = PS_RS.tile([1, HALF], F32, tag="rs", name=f"rs{t}")
                pv_t[t] = PS_PV.tile([128, HALF], F32, tag="pv", name=f"pv{t}")
            at = at_t.pop((t, m))
            nc.tensor.matmul(rs_t[t], ones, at, start=(m == 0), stop=(m == 7))
            nc.tensor.matmul(pv_t[t], v_sb[m][:, hs], at,
                             start=(m == 0), stop=(m == 7))

        from concourse.dve_ops import (
            RECIP_APPROX_FAST_CONSTS,
            RECIPROCAL_APPROX_FAST,
        )

        def recip(t):
            # softmax denominators: 1/rowsum, approx (~18 good bits), f32r
            # out so the broadcast matmul can consume it directly
            rc = RC.tile([1, HALF], F32R, tag="rc", name=f"rc{t}")
            cc = RECIP_APPROX_FAST_CONSTS
            nc.vector._custom_dve(RECIPROCAL_APPROX_FAST, out=rc,
                                  in0=rs_t.pop(t), s0=cc["s0"], s1=cc["s1"],
                                  imm2=cc["imm2"])
            rc_t[t] = rc

        def bcp_mm(t):
            # partition-broadcast recip via contraction-1 matmul (no DMA)
            bcp = PS_BC.tile([128, HALF], F32, tag="bcp", name=f"bcp{t}")
            nc.tensor.matmul(bcp, ones_row, rc_t.pop(t),
                             start=True, stop=True)
            bcs = BCS.tile([128, HALF], F32, tag="bcs", name=f"bcs{t}")
            nc.scalar.activation(bcs, bcp, func=IDENT)
            bcp_t[t] = bcs

        def oh_mul(t):
            oh = OH.tile([128, HALF], F32R, tag="oh", name=f"oh{t}")
            nc.vector.tensor_mul(oh, pv_t.pop(t), bcp_t.pop(t))
            oh_t[t] = oh

        def proj_acc(t):
            h, i = divmod(t, 2)
            ns = slice(i * HALF, (i + 1) * HALF)
            pj = PS_PJ.tile([128, HALF], F32, tag="pj", name=f"pj{t}")
            nc.tensor.matmul(pj, pw_sb[:, h, :], oh_t.pop(t),
                             start=True, stop=True)
            if h == 0:
                nc.vector.tensor_copy(yacc[:, ns], pj)
            else:
                nc.vector.tensor_add(yacc[:, ns], yacc[:, ns], pj)

        def finalize(i):
            ns = slice(i * HALF, (i + 1) * HALF)
            nc.scalar.activation(yt_sb[:, ns], yacc[:, ns], func=IDENT,
                                 bias=pb_sb)
            nc.sync.dma_start(out=yT[:, ns], in_=yt_sb[:, ns])

        pi = 0
        for t in range(NBLK + 2):
            quota = piece_quota.get(t, 0)
            for m in range(8):
                if t < NBLK:
                    s_exp_mul(t, m)
                if 1 <= t <= NBLK:
                    ones_pv(t - 1, m)
                if m == 2 and 2 <= t:
                    bcp_mm(t - 2)    # recip(t-2) done by now; 213ns on PE
                if m == 4 and 2 <= t:
                    oh_mul(t - 2)    # bcp just above; frees pv(t-2)
                if quota and m % (8 // quota) == (8 // quota) - 1:
                    pieces[pi](); pi += 1
            if 1 <= t <= NBLK:
                recip(t - 1)         # rs(t-1) just stopped
            if 2 <= t:
                proj_acc(t - 2)      # PE reaches this after the block's work
                if t - 2 >= NBLK - 2:
                    finalize((t - 2) % 2)
        assert pi == len(pieces)


_CACHE = {}


def _prep_inputs(x, B_bias, wq_w, wq_b, wk_w, wk_b, wv_w, wv_b, proj_w, proj_b):
    s = 1.0 / math.sqrt(DH)
    f = np.float32
    xTh = np.ascontiguousarray(x.transpose(0, 2, 1)).astype(f)      # [8,256,1024]
    ebh = np.ascontiguousarray(np.exp(np.asarray(B_bias, np.float32).T))
    wq_s = (np.asarray(wq_w) * s).astype(f)
    wqb_t = np.ascontiguousarray((np.asarray(wq_b) * s).reshape(8, 128).T)
    wkb_t = np.ascontiguousarray(np.asarray(wk_b, f).reshape(8, 128).T)
    wvbb = np.ascontiguousarray(np.broadcast_to(np.asarray(wv_b, f), (128, C)))
    pb_t = np.ascontiguousarray(np.asarray(proj_b, f).reshape(128, 1))
    shared = dict(eb=ebh, wq=wq_s, wk=np.asarray(wk_w, f),
                  wv=np.asarray(wv_w, f), wqb=wqb_t, wkb=wkb_t, wvbb=wvbb,
                  pw=np.asarray(proj_w, f), pb=pb_t)
    return [dict(shared, xT=xTh[b]) for b in range(NCORES)]


def kernel(**inputs):
    from concourse.bass_utils import run_bass_kernel_spmd

    if "nc" not in _CACHE:
        _CACHE["nc"] = build_nc()
    nc = _CACHE["nc"]
    in_maps = _prep_inputs(**inputs)
    res = run_bass_kernel_spmd(nc, in_maps, core_ids=list(range(NCORES)))
    out = np.stack([np.asarray(res.results[b]["yT"]).T for b in range(NCORES)])
    return np.ascontiguousarray(out.astype(np.float32))
